# revision 8
# baseline (speedup 1.0000x reference)
"""Chamfer distance L2 kernel for Trainium2 (8 NeuronCores).

Problem: xyz1 [4, 8192, 3] f32, xyz2 [4, 8192, 3] f32.
Outputs: dist1 [4, 8192] (min_j ||xyz1[b,i]-xyz2[b,j]||^2),
         dist2 [4, 8192] (min_i over xyz1 for each xyz2 point).

Sharding: 4 batches x 2 directions = 8 independent jobs, one per core.
Each core: queries q [8192,3], refs r [8192,3] -> dist [8192].

Per-core algorithm:
  d_ij = sq_i + sq_j - 2 q_i . r_j  computed on the PE as a K=24 matmul:
  each fp32 value is split into 3 bf16 terms (h+m+l); the 6 dominant
  cross products per coordinate (hh, hm, mh, hl, lh, mm) plus 3-term
  splits of the two squared norms give fp32-grade accuracy at full bf16
  PE speed (fp32 matmul would be 4x slower).
  The [128 x 512] PSUM tiles (grouped into [128 x 2048] chunks) are
  min-reduced on the DVE with tensor_tensor_reduce, whose scalar/accum
  [128,1] operands chain the running min with no extra reduction pass.
"""

import sys

for _p in ("/opt/trn_rl_repo", "/root/.axon_site/_ro/trn_rl_repo"):
    if _p not in sys.path:
        sys.path.insert(0, _p)

import numpy as np

import concourse.bacc as bacc
import concourse.bass as bass
import concourse.mybir as mybir
from concourse.bass_utils import run_bass_kernel_spmd
from concourse.masks import make_identity
from concourse.tile import TileContext

B = 4
N = 8192          # points per cloud
P = 128           # partitions
NQT = N // P      # 64 query tiles
CHUNK = 2048      # refs per consumer chunk (4 PSUM banks)
NCHUNK = N // CHUNK
MM_N = 512        # matmul moving free dim (1 PSUM bank fp32)
K = 24            # contraction rows after 3-term bf16 split
BIG = 3.0e38      # +inf stand-in for min chains

F32 = mybir.dt.float32
BF16 = mybir.dt.bfloat16


def _split3(nc, pool, src_f32, shape):
    """3-term bf16 split of an f32 tile: src ~= h + m + l. Returns bf16 tiles."""
    h = pool.tile(shape, BF16, tag=f"h{shape[-1]}")
    m = pool.tile(shape, BF16, tag=f"m{shape[-1]}")
    l = pool.tile(shape, BF16, tag=f"l{shape[-1]}")
    r1 = pool.tile(shape, F32, tag=f"r1_{shape[-1]}")
    r2 = pool.tile(shape, F32, tag=f"r2_{shape[-1]}")
    nc.vector.tensor_copy(h, src_f32)
    nc.vector.tensor_sub(r1, src_f32, h)
    nc.vector.tensor_copy(m, r1)
    nc.vector.tensor_sub(r2, r1, m)
    nc.vector.tensor_copy(l, r2)
    return h, m, l


def _build_augT(nc, tc, ctx_pools, dram_pts, is_query, augT, identity, pro_sbuf, pro_psum):
    """Build the K-major [24, 8192] bf16 augmented matrix for one point set.

    Query rows pair with ref rows so that sum_k q_k*r_k = sq_q + sq_r - 2 q.r.
    """
    # Load points: DRAM [8192, 3] -> SBUF [128, 64, 3] f32 (point = n*128 + p)
    xyz = pro_sbuf.tile([P, NQT, 3], F32, tag="xyz")
    nc.sync.dma_start(out=xyz, in_=dram_pts.rearrange("(n p) d -> p n d", p=P))

    # Squared norms [128, 64]
    sqt = pro_sbuf.tile([P, NQT, 3], F32, tag="sqt")
    nc.vector.tensor_mul(sqt, xyz, xyz)
    sq = pro_sbuf.tile([P, NQT], F32, tag="sq")
    nc.vector.tensor_reduce(sq, sqt, axis=mybir.AxisListType.X, op=mybir.AluOpType.add)

    # Coordinate basis: queries use -2*x, refs use y as-is
    base = pro_sbuf.tile([P, NQT, 3], F32, tag="base")
    if is_query:
        nc.scalar.mul(base, xyz, -2.0)
    else:
        nc.scalar.copy(base, xyz)

    ch, cm, cl = _split3(nc, pro_sbuf, base, [P, NQT, 3])
    sh, sm, sl = _split3(nc, pro_sbuf, sq, [P, NQT])

    # Assemble [128, 64, 24] bf16 aug matrix
    aug = pro_sbuf.tile([P, NQT, K], BF16, tag="aug")
    if is_query:
        coord_rows = [(0, ch), (1, ch), (2, cm), (3, ch), (4, cl), (5, cm)]
    else:
        coord_rows = [(0, ch), (1, cm), (2, ch), (3, cl), (4, ch), (5, cm)]
    for c in range(3):
        for off, srcs in coord_rows:
            nc.vector.tensor_copy(aug[:, :, 6 * c + off], srcs[:, :, c])
    sq_base = 18 if is_query else 21
    one_base = 21 if is_query else 18
    for off, srcs in ((0, sh), (1, sm), (2, sl)):
        nc.vector.tensor_copy(aug[:, :, sq_base + off], srcs)
    nc.vector.memset(aug[:, :, one_base:one_base + 3], 1.0)

    # Transpose 64 chunks of [128, 24] -> [24, 128] into PSUM, copy to augT
    per_round = 16
    for rnd in range(NQT // per_round):
        pst = pro_psum.tile([K, per_round * P], BF16, tag="pst")
        for j in range(per_round):
            c = rnd * per_round + j
            nc.tensor.transpose(pst[:, j * P:(j + 1) * P], aug[:, c, :], identity)
        nc.scalar.copy(augT[:, rnd * per_round * P:(rnd + 1) * per_round * P], pst)


def build_program():
    nc = bacc.Bacc("TRN2", target_bir_lowering=False, debug=False)
    q_dram = nc.dram_tensor("q", [N, 3], F32, kind="ExternalInput").ap()
    r_dram = nc.dram_tensor("r", [N, 3], F32, kind="ExternalInput").ap()
    out_dram = nc.dram_tensor("dist", [N], F32, kind="ExternalOutput").ap()

    with TileContext(nc) as tc:
        from contextlib import ExitStack
        with ExitStack() as ctx:
            consts = ctx.enter_context(tc.tile_pool(name="consts", bufs=1))
            identity = consts.tile([P, P], BF16)
            make_identity(nc, identity)
            identity_f32 = consts.tile([P, P], F32)
            make_identity(nc, identity_f32)
            augT_q = consts.tile([K, N], BF16)
            augT_r = consts.tile([K, N], BF16)
            dist_sb = consts.tile([P, NQT], F32)

            # ---- prologue: build both K-major aug matrices ----
            with tc.tile_pool(name="pro_sbuf", bufs=2) as pro_sbuf, \
                 tc.tile_pool(name="pro_psum", bufs=2, space="PSUM") as pro_psum:
                _build_augT(nc, tc, None, q_dram, True, augT_q, identity, pro_sbuf, pro_psum)
                _build_augT(nc, tc, None, r_dram, False, augT_r, identity, pro_sbuf, pro_psum)

            # ---- main loop ----
            QG = 8  # query tiles per partial-min group
            with tc.tile_pool(name="mm_psum", bufs=2, space="PSUM") as mm_psum, \
                 tc.tile_pool(name="parts", bufs=2) as part_pool:
                for qtg in range(NQT // QG):
                    partial = part_pool.tile([P, QG, NCHUNK], F32, tag="part")
                    for qi in range(QG):
                        qt = qtg * QG + qi
                        lhsT = augT_q[:, qt * P:(qt + 1) * P]
                        for ch in range(NCHUNK):
                            ps = mm_psum.tile([P, CHUNK], F32, tag="ps")
                            for j in range(CHUNK // MM_N):
                                col = ch * CHUNK + j * MM_N
                                nc.tensor.matmul(
                                    ps[:, j * MM_N:(j + 1) * MM_N],
                                    lhsT,
                                    augT_r[:, col:col + MM_N],
                                    start=True,
                                    stop=True,
                                )
                            nc.vector.tensor_reduce(
                                partial[:, qi, ch:ch + 1],
                                ps,
                                axis=mybir.AxisListType.X,
                                op=mybir.AluOpType.min,
                            )
                    nc.vector.tensor_reduce(
                        dist_sb[:, qtg * QG:(qtg + 1) * QG],
                        partial,
                        axis=mybir.AxisListType.X,
                        op=mybir.AluOpType.min,
                    )

            # ---- epilogue: transpose [128, 64] -> [64, 128], DMA out ----
            with tc.tile_pool(name="ep_psum", bufs=1, space="PSUM") as ep_psum, \
                 tc.tile_pool(name="ep_sbuf", bufs=1) as ep_sbuf:
                pst = ep_psum.tile([NQT, P], F32)
                nc.tensor.transpose(pst, dist_sb, identity_f32)
                osb = ep_sbuf.tile([NQT, P], F32)
                # true min squared distances are >= 0; the expansion formula
                # can go slightly negative for near-duplicate points
                nc.vector.tensor_scalar_max(osb, pst, 0.0)
                nc.sync.dma_start(out=out_dram.rearrange("(a b) -> a b", b=P), in_=osb)

    nc.compile()
    return nc


_NC_CACHE = None


def _get_program():
    global _NC_CACHE
    if _NC_CACHE is None:
        _NC_CACHE = build_program()
    return _NC_CACHE


def kernel(xyz1: np.ndarray, xyz2: np.ndarray):
    xyz1 = np.ascontiguousarray(np.asarray(xyz1, dtype=np.float32))
    xyz2 = np.ascontiguousarray(np.asarray(xyz2, dtype=np.float32))
    nc = _get_program()
    in_maps = []
    for b in range(B):
        in_maps.append({"q": xyz1[b], "r": xyz2[b]})
        in_maps.append({"q": xyz2[b], "r": xyz1[b]})
    res = run_bass_kernel_spmd(nc, in_maps, core_ids=list(range(2 * B)))
    dist1 = np.stack([np.asarray(res.results[2 * b]["dist"]) for b in range(B)])
    dist2 = np.stack([np.asarray(res.results[2 * b + 1]["dist"]) for b in range(B)])
    return dist1, dist2


# revision 14
# speedup vs baseline: 209.9154x; 209.9154x over previous
"""Chamfer distance L2 kernel for Trainium2 (8 NeuronCores).

Problem: xyz1 [4, 8192, 3] f32, xyz2 [4, 8192, 3] f32.
Outputs: dist1 [4, 8192] (min_j ||xyz1[b,i]-xyz2[b,j]||^2),
         dist2 [4, 8192] (min_i over xyz1 for each xyz2 point).

Sharding: 4 batches x 2 directions = 8 independent jobs, one per core.
Each core: queries q [8192,3], refs r [8192,3] -> dist [8192].

Per-core algorithm:
  d_ij = sq_i + sq_j - 2 q_i . r_j  computed on the PE as a K=24 matmul:
  each fp32 value is split into 3 bf16 terms (h+m+l); the 6 dominant
  cross products per coordinate (hh, hm, mh, hl, lh, mm) plus 3-term
  splits of the two squared norms give fp32-grade accuracy at full bf16
  PE speed (fp32 matmul would be 4x slower).
  Consumers are balanced across two engines: per query tile, chunk 0 of
  the PSUM distance row is pair-min-consumed by the DVE directly, chunks
  1-3 are copied PSUM->SBUF fp16 by the ACT engine while the DVE folds
  them at its 2x 16-bit rate; final reduces are grouped 8 query tiles at
  a time to amortize the 1x tensor_reduce.
"""

import sys

for _p in ("/opt/trn_rl_repo", "/root/.axon_site/_ro/trn_rl_repo"):
    if _p not in sys.path:
        sys.path.insert(0, _p)

import numpy as np

import concourse.bacc as bacc
import concourse.bass as bass
import concourse.mybir as mybir
from concourse.bass_utils import run_bass_kernel_spmd
from concourse.masks import make_identity
from concourse.tile import TileContext

B = 4
N = 8192          # points per cloud
P = 128           # partitions
NQT = N // P      # 64 query tiles
CHUNK = 2048      # refs per consumer chunk (4 PSUM banks)
NCHUNK = N // CHUNK
MM_N = 512        # matmul moving free dim (1 PSUM bank fp32)
K = 24            # contraction rows after 3-term bf16 split
BIG = 3.0e38      # +inf stand-in for min chains

F32 = mybir.dt.float32
BF16 = mybir.dt.bfloat16
F16 = mybir.dt.float16


def _split3(nc, pool, src_f32, shape):
    """3-term bf16 split of an f32 tile: src ~= h + m + l. Returns bf16 tiles."""
    h = pool.tile(shape, BF16, tag=f"h{shape[-1]}")
    m = pool.tile(shape, BF16, tag=f"m{shape[-1]}")
    l = pool.tile(shape, BF16, tag=f"l{shape[-1]}")
    r1 = pool.tile(shape, F32, tag=f"r1_{shape[-1]}")
    r2 = pool.tile(shape, F32, tag=f"r2_{shape[-1]}")
    nc.vector.tensor_copy(h, src_f32)
    nc.vector.tensor_sub(r1, src_f32, h)
    nc.vector.tensor_copy(m, r1)
    nc.vector.tensor_sub(r2, r1, m)
    nc.vector.tensor_copy(l, r2)
    return h, m, l


def _build_augT(nc, tc, ctx_pools, dram_pts, is_query, augT, identity, pro_sbuf, pro_psum):
    """Build the K-major [24, 8192] bf16 augmented matrix for one point set.

    Query rows pair with ref rows so that sum_k q_k*r_k = sq_q + sq_r - 2 q.r.
    """
    # Load points: DRAM [8192, 3] -> SBUF [128, 64, 3] f32 (point = n*128 + p)
    xyz = pro_sbuf.tile([P, NQT, 3], F32, tag="xyz")
    nc.sync.dma_start(out=xyz, in_=dram_pts.rearrange("(n p) d -> p n d", p=P))

    # Squared norms [128, 64]
    sqt = pro_sbuf.tile([P, NQT, 3], F32, tag="sqt")
    nc.vector.tensor_mul(sqt, xyz, xyz)
    sq = pro_sbuf.tile([P, NQT], F32, tag="sq")
    nc.vector.tensor_reduce(sq, sqt, axis=mybir.AxisListType.X, op=mybir.AluOpType.add)

    # Coordinate basis: queries use -2*x, refs use y as-is
    base = pro_sbuf.tile([P, NQT, 3], F32, tag="base")
    if is_query:
        nc.scalar.mul(base, xyz, -2.0)
    else:
        nc.scalar.copy(base, xyz)

    ch, cm, cl = _split3(nc, pro_sbuf, base, [P, NQT, 3])
    sh, sm, sl = _split3(nc, pro_sbuf, sq, [P, NQT])

    # Assemble [128, 64, 24] bf16 aug matrix
    aug = pro_sbuf.tile([P, NQT, K], BF16, tag="aug")
    if is_query:
        coord_rows = [(0, ch), (1, ch), (2, cm), (3, ch), (4, cl), (5, cm)]
    else:
        coord_rows = [(0, ch), (1, cm), (2, ch), (3, cl), (4, ch), (5, cm)]
    for c in range(3):
        for off, srcs in coord_rows:
            nc.vector.tensor_copy(aug[:, :, 6 * c + off], srcs[:, :, c])
    sq_base = 18 if is_query else 21
    one_base = 21 if is_query else 18
    for off, srcs in ((0, sh), (1, sm), (2, sl)):
        nc.vector.tensor_copy(aug[:, :, sq_base + off], srcs)
    nc.vector.memset(aug[:, :, one_base:one_base + 3], 1.0)

    # Transpose 64 chunks of [128, 24] -> [24, 128] into PSUM, copy to augT
    per_round = 16
    for rnd in range(NQT // per_round):
        pst = pro_psum.tile([K, per_round * P], BF16, tag="pst")
        for j in range(per_round):
            c = rnd * per_round + j
            nc.tensor.transpose(pst[:, j * P:(j + 1) * P], aug[:, c, :], identity)
        nc.scalar.copy(augT[:, rnd * per_round * P:(rnd + 1) * per_round * P], pst)


def build_program():
    nc = bacc.Bacc("TRN2", target_bir_lowering=False, debug=False)
    q_dram = nc.dram_tensor("q", [N, 3], F32, kind="ExternalInput").ap()
    r_dram = nc.dram_tensor("r", [N, 3], F32, kind="ExternalInput").ap()
    out_dram = nc.dram_tensor("dist", [N], F32, kind="ExternalOutput").ap()

    with TileContext(nc) as tc:
        from contextlib import ExitStack
        with ExitStack() as ctx:
            consts = ctx.enter_context(tc.tile_pool(name="consts", bufs=1))
            identity = consts.tile([P, P], BF16)
            make_identity(nc, identity)
            identity_f32 = consts.tile([P, P], F32)
            make_identity(nc, identity_f32)
            augT_q = consts.tile([K, N], BF16)
            augT_r = consts.tile([K, N], BF16)
            dist_sb = consts.tile([P, NQT], F32)

            # ---- prologue: build both K-major aug matrices ----
            with tc.tile_pool(name="pro_sbuf", bufs=2) as pro_sbuf, \
                 tc.tile_pool(name="pro_psum", bufs=2, space="PSUM") as pro_psum:
                _build_augT(nc, tc, None, q_dram, True, augT_q, identity, pro_sbuf, pro_psum)
                _build_augT(nc, tc, None, r_dram, False, augT_r, identity, pro_sbuf, pro_psum)

            # ---- main loop ----
            # Per query tile: 4 PSUM chunks of 2048 refs. Chunk 0 is
            # min-reduced by the DVE straight from PSUM (1x). Chunks 1-3 are
            # copied PSUM->SBUF bf16 by the ACT engine (1x, in parallel) and
            # the DVE folds those at its 2x bf16 rate — balancing the two
            # engines instead of serializing everything through the DVE.
            MIN = mybir.AluOpType.min
            X = mybir.AxisListType.X
            H = CHUNK // 2
            with tc.tile_pool(name="mm_psum", bufs=2, space="PSUM") as mm_psum, \
                 tc.tile_pool(name="stage", bufs=2) as stage_pool, \
                 tc.tile_pool(name="small", bufs=4) as small_pool:
                for qt in range(NQT):
                    lhsT = augT_q[:, qt * P:(qt + 1) * P]
                    stage = stage_pool.tile([P, 3, CHUNK], F16, tag="stage")
                    partA = small_pool.tile([P, 1], F32, tag="partA")
                    for ch in range(NCHUNK):
                        ps = mm_psum.tile([P, CHUNK], F32, tag="ps")
                        for j in range(CHUNK // MM_N):
                            col = ch * CHUNK + j * MM_N
                            nc.tensor.matmul(
                                ps[:, j * MM_N:(j + 1) * MM_N],
                                lhsT,
                                augT_r[:, col:col + MM_N],
                                start=True,
                                stop=True,
                            )
                        if ch == 0:
                            # DVE min-reduces this chunk straight from PSUM
                            nc.vector.tensor_reduce(partA, ps, axis=X, op=MIN)
                        else:
                            # ACT copies to fp16 SBUF for 2x DVE folds
                            nc.scalar.copy(stage[:, ch - 1, :], ps)
                    # fp16 fold tree on DVE (2x mode, SBUF step-1)
                    m1 = stage_pool.tile([P, CHUNK], F16, tag="m1")
                    nc.vector.tensor_tensor(m1, stage[:, 0, :], stage[:, 1, :], op=MIN)
                    f1 = stage_pool.tile([P, H], F16, tag="f1")
                    nc.vector.tensor_tensor(f1, m1[:, :H], m1[:, H:], op=MIN)
                    g1 = stage_pool.tile([P, H], F16, tag="g1")
                    nc.vector.tensor_tensor(
                        g1, stage[:, 2, :H], stage[:, 2, H:], op=MIN)
                    f2 = stage_pool.tile([P, H], F16, tag="f2")
                    nc.vector.tensor_tensor(f2, f1, g1, op=MIN)
                    partB = small_pool.tile([P, 1], F32, tag="partB")
                    nc.vector.tensor_reduce(partB, f2, axis=X, op=MIN)
                    nc.vector.tensor_tensor(dist_sb[:, qt:qt + 1], partA, partB, op=MIN)

            # ---- epilogue: transpose [128, 64] -> [64, 128], DMA out ----
            with tc.tile_pool(name="ep_psum", bufs=1, space="PSUM") as ep_psum, \
                 tc.tile_pool(name="ep_sbuf", bufs=1) as ep_sbuf:
                pst = ep_psum.tile([NQT, P], F32)
                nc.tensor.transpose(pst, dist_sb, identity_f32)
                osb = ep_sbuf.tile([NQT, P], F32)
                # true min squared distances are >= 0; the expansion formula
                # can go slightly negative for near-duplicate points
                nc.vector.tensor_scalar_max(osb, pst, 0.0)
                nc.sync.dma_start(out=out_dram.rearrange("(a b) -> a b", b=P), in_=osb)

    nc.compile()
    return nc


_NC_CACHE = None


def _get_program():
    global _NC_CACHE
    if _NC_CACHE is None:
        _NC_CACHE = build_program()
    return _NC_CACHE


def kernel(xyz1: np.ndarray, xyz2: np.ndarray):
    xyz1 = np.ascontiguousarray(np.asarray(xyz1, dtype=np.float32))
    xyz2 = np.ascontiguousarray(np.asarray(xyz2, dtype=np.float32))
    nc = _get_program()
    in_maps = []
    for b in range(B):
        in_maps.append({"q": xyz1[b], "r": xyz2[b]})
        in_maps.append({"q": xyz2[b], "r": xyz1[b]})
    res = run_bass_kernel_spmd(nc, in_maps, core_ids=list(range(2 * B)))
    dist1 = np.stack([np.asarray(res.results[2 * b]["dist"]) for b in range(B)])
    dist2 = np.stack([np.asarray(res.results[2 * b + 1]["dist"]) for b in range(B)])
    return dist1, dist2


# revision 23
# speedup vs baseline: 222.1547x; 1.0583x over previous
"""Chamfer distance L2 kernel for Trainium2 (8 NeuronCores).

Problem: xyz1 [4, 8192, 3] f32, xyz2 [4, 8192, 3] f32.
Outputs: dist1 [4, 8192] (min_j ||xyz1[b,i]-xyz2[b,j]||^2),
         dist2 [4, 8192] (min_i over xyz1 for each xyz2 point).

Sharding: 4 batches x 2 directions = 8 independent jobs, one per core.
Each core: queries q [8192,3], refs r [8192,3] -> dist [8192].

Per-core algorithm:
  d_ij = sq_i + sq_j - 2 q_i . r_j  computed on the PE as a K=24 matmul:
  each fp32 value is split into 3 bf16 terms (h+m+l); the 6 dominant
  cross products per coordinate (hh, hm, mh, hl, lh, mm) plus 3-term
  splits of the two squared norms give fp32-grade accuracy at full bf16
  PE speed (fp32 matmul would be 4x slower). The K-major [24, 8192]
  bf16 augmented layouts are precomputed on the host (cheap O(N) prep),
  so the device runs no prologue beyond two contiguous DMAs.
  Consumers are balanced across two engines: per query tile, chunk 0 of
  the PSUM distance row is min-reduced by the DVE directly from PSUM,
  chunks 1-3 are copied PSUM->SBUF fp16 by the ACT engine while the DVE
  folds them at its 2x 16-bit rate.
"""

import sys

for _p in ("/opt/trn_rl_repo", "/root/.axon_site/_ro/trn_rl_repo"):
    if _p not in sys.path:
        sys.path.insert(0, _p)

import ml_dtypes
import numpy as np

import concourse.bacc as bacc
import concourse.mybir as mybir
from concourse.bass_utils import run_bass_kernel_spmd
from concourse.masks import make_identity
from concourse.tile import TileContext

B = 4
N = 8192          # points per cloud
P = 128           # partitions
NQT = N // P      # 64 query tiles
CHUNK = 2048      # refs per consumer chunk (4 PSUM banks)
NCHUNK = N // CHUNK
MM_N = 512        # matmul moving free dim (1 PSUM bank fp32)
K = 24            # contraction rows after 3-term bf16 split

F32 = mybir.dt.float32
BF16 = mybir.dt.bfloat16
F16 = mybir.dt.float16

BF = ml_dtypes.bfloat16


def _split3_np(x):
    """3-term bf16 split: x ~= h + m + l (all returned as fp32 arrays)."""
    h = x.astype(BF).astype(np.float32)
    r1 = x - h
    m = r1.astype(BF).astype(np.float32)
    r2 = r1 - m
    l = r2.astype(BF).astype(np.float32)
    return h, m, l


def _build_aug_np(pts, is_query):
    """Host-side K-major augmented layout [24, 8192] bf16.

    Row k of the query layout pairs with row k of the ref layout so that
    sum_k q_k * r_k = sq_q + sq_r - 2 q.r  (to ~fp32 accuracy).
    """
    pts = np.asarray(pts, dtype=np.float32)
    sq = (pts * pts).sum(-1)                      # [N]
    base = (-2.0 * pts) if is_query else pts
    ch, cm, cl = _split3_np(base)                 # [N, 3] each
    sh, sm, sl = _split3_np(sq)                   # [N]
    ones = np.ones_like(sq)
    rows = []
    for c in range(3):
        if is_query:
            rows += [ch[:, c], ch[:, c], cm[:, c], ch[:, c], cl[:, c], cm[:, c]]
        else:
            rows += [ch[:, c], cm[:, c], ch[:, c], cl[:, c], ch[:, c], cm[:, c]]
    if is_query:
        rows += [sh, sm, sl, ones, ones, ones]
    else:
        rows += [ones, ones, ones, sh, sm, sl]
    return np.ascontiguousarray(np.stack(rows, 0).astype(BF))


def build_program():
    nc = bacc.Bacc("TRN2", target_bir_lowering=False, debug=False)
    aq_dram = nc.dram_tensor("aq", [K, N], BF16, kind="ExternalInput").ap()
    ar_dram = nc.dram_tensor("ar", [K, N], BF16, kind="ExternalInput").ap()
    out_dram = nc.dram_tensor("dist", [N], F32, kind="ExternalOutput").ap()

    with TileContext(nc) as tc:
        from contextlib import ExitStack
        with ExitStack() as ctx:
            consts = ctx.enter_context(tc.tile_pool(name="consts", bufs=1))
            identity_f32 = consts.tile([P, P], F32)
            make_identity(nc, identity_f32)
            augT_q = consts.tile([K, N], BF16)
            augT_r = consts.tile([K, N], BF16)
            dist_sb = consts.tile([P, NQT], F32)
            # contiguous row-major loads; each partition gets a 16KB stream
            nc.sync.dma_start(out=augT_q, in_=aq_dram)
            nc.sync.dma_start(out=augT_r, in_=ar_dram)

            # ---- main loop ----
            # Per query tile: 4 PSUM chunks of 2048 refs. Chunk 0 is
            # min-reduced by the DVE straight from PSUM (1x). Chunks 1-3 are
            # copied PSUM->SBUF fp16 by the ACT engine (1x, in parallel) and
            # the DVE folds those at its 2x fp16 rate — balancing the two
            # engines instead of serializing everything through the DVE.
            MIN = mybir.AluOpType.min
            X = mybir.AxisListType.X
            H = CHUNK // 2
            with tc.tile_pool(name="mm_psum", bufs=2, space="PSUM") as mm_psum, \
                 tc.tile_pool(name="stage", bufs=3, space="SBUF") as stage_pool, \
                 tc.tile_pool(name="small", bufs=8) as small_pool:
                for qt in range(NQT):
                    lhsT = augT_q[:, qt * P:(qt + 1) * P]
                    stage = stage_pool.tile([P, 3, CHUNK], F16, tag="stage")
                    partA = small_pool.tile([P, 1], F32, tag="partA")
                    for ch in range(NCHUNK):
                        ps = mm_psum.tile([P, CHUNK], F32, tag="ps")
                        for j in range(CHUNK // MM_N):
                            col = ch * CHUNK + j * MM_N
                            nc.tensor.matmul(
                                ps[:, j * MM_N:(j + 1) * MM_N],
                                lhsT,
                                augT_r[:, col:col + MM_N],
                                start=True,
                                stop=True,
                            )
                        if ch == 0:
                            # DVE min-reduces this chunk straight from PSUM
                            nc.vector.tensor_reduce(partA, ps, axis=X, op=MIN)
                        else:
                            # ACT copies to fp16 SBUF for 2x DVE folds
                            nc.scalar.copy(stage[:, ch - 1, :], ps)
                    # fp16 fold tree on DVE (2x mode, SBUF step-1)
                    m1 = stage_pool.tile([P, CHUNK], F16, tag="m1")
                    nc.vector.tensor_tensor(m1, stage[:, 0, :], stage[:, 1, :], op=MIN)
                    f1 = stage_pool.tile([P, H], F16, tag="f1")
                    nc.vector.tensor_tensor(f1, m1[:, :H], m1[:, H:], op=MIN)
                    g1 = stage_pool.tile([P, H], F16, tag="g1")
                    nc.vector.tensor_tensor(
                        g1, stage[:, 2, :H], stage[:, 2, H:], op=MIN)
                    f2 = stage_pool.tile([P, H], F16, tag="f2")
                    nc.vector.tensor_tensor(f2, f1, g1, op=MIN)
                    f3 = stage_pool.tile([P, H // 2], F16, tag="f3")
                    nc.vector.tensor_tensor(f3, f2[:, :H // 2], f2[:, H // 2:], op=MIN)
                    partB = small_pool.tile([P, 1], F32, tag="partB")
                    nc.vector.tensor_reduce(partB, f3, axis=X, op=MIN)
                    nc.vector.tensor_tensor(dist_sb[:, qt:qt + 1], partA, partB, op=MIN)

            # ---- epilogue: transpose [128, 64] -> [64, 128], DMA out ----
            with tc.tile_pool(name="ep_psum", bufs=1, space="PSUM") as ep_psum, \
                 tc.tile_pool(name="ep_sbuf", bufs=1) as ep_sbuf:
                pst = ep_psum.tile([NQT, P], F32)
                nc.tensor.transpose(pst, dist_sb, identity_f32)
                osb = ep_sbuf.tile([NQT, P], F32)
                # true min squared distances are >= 0; the expansion formula
                # can go slightly negative for near-duplicate points
                nc.vector.tensor_scalar_max(osb, pst, 0.0)
                nc.sync.dma_start(out=out_dram.rearrange("(a b) -> a b", b=P), in_=osb)

    nc.compile()
    return nc


_NC_CACHE = None


def _get_program():
    global _NC_CACHE
    if _NC_CACHE is None:
        _NC_CACHE = build_program()
    return _NC_CACHE


def kernel(xyz1: np.ndarray, xyz2: np.ndarray):
    xyz1 = np.ascontiguousarray(np.asarray(xyz1, dtype=np.float32))
    xyz2 = np.ascontiguousarray(np.asarray(xyz2, dtype=np.float32))
    nc = _get_program()
    in_maps = []
    for b in range(B):
        aq1 = _build_aug_np(xyz1[b], True)
        ar2 = _build_aug_np(xyz2[b], False)
        aq2 = _build_aug_np(xyz2[b], True)
        ar1 = _build_aug_np(xyz1[b], False)
        in_maps.append({"aq": aq1, "ar": ar2})   # dist1[b]
        in_maps.append({"aq": aq2, "ar": ar1})   # dist2[b]
    res = run_bass_kernel_spmd(nc, in_maps, core_ids=list(range(2 * B)))
    dist1 = np.stack([np.asarray(res.results[2 * b]["dist"]) for b in range(B)])
    dist2 = np.stack([np.asarray(res.results[2 * b + 1]["dist"]) for b in range(B)])
    return dist1, dist2
